# revision 43
# baseline (speedup 1.0000x reference)
"""Multi-head attention on 8 Trainium2 NeuronCores — ACT-paced fp16 pipeline.

Sharding: tensor-parallel over heads (2 heads/core), full batch on every
core; host sums the 8 partial outputs and adds b_o + b_v @ w_o.

Key structure (vs the previous version): the scalar engine's exp is the
irreducible floor (~73 us/batch at N=1024 per ACTIVATE), so the whole
kernel is paced by it and every other engine hides underneath:

  - Scores: 2-head row-tiled matmul pairs (tile_position (0,0)/(64,0))
    into one [128, 1024] f32 PSUM pair-tile (h0 cols 0:512 -> bank A,
    h1 cols 512:1024 -> bank B), double-buffered. One exp per kt step
    covers both heads.
  - b_k is dropped: softmax is invariant to per-q constants, and the
    only score term that varies over k is (q + b_q)@k. b_q is added to
    qT at evacuation (exact same math as the reference).
  - Softmax denominators: av row 64 (ones-row trick) -> DVE reciprocal
    [1, 512] -> gpsimd partition_broadcast -> attnT is normalized once
    at evacuation. Stage D needs no per-unit scaling.
  - Stage D per (tt, half): two concurrent row-tiled MMs -> one DVE
    tensor_add (cast fp16) -> DMA out.
  - Stage B of batch bi+1 (projections) is interleaved into stage C of
    bi via a work queue, so ACT never idles at batch boundaries.

PSUM: stp 2x2 banks + av 2x1 + scratch 2x1 (po pairs / ppr / pv) = 8.
"""

import numpy as np

import concourse.bacc as bacc
import concourse.mybir as mybir
from concourse.tile import TileContext
from concourse import bass_utils

dt = mybir.dt
F32 = dt.float32
F16 = dt.float16
AF = mybir.ActivationFunctionType
ALU = mybir.AluOpType

B, S, D = 4, 2048, 1024
H, DH = 16, 64
NCORES = 8
HPC = H // NCORES          # heads per core = 2
DHC = HPC * DH             # 128 projection cols per core

_CACHE = {}


def build_nc(b=B, s=S):
    d = D
    n_tt = s // 128            # 128-token tiles (k tiles, and q out-tiles)
    n_kt = d // 128            # contraction tiles for projections
    qw = 512                   # q-chunk width in stage C
    n_jc = s // qw
    assert s % 512 == 0 and d == 1024

    nc = bacc.Bacc("TRN2", target_bir_lowering=False, debug=False)

    x_d = nc.dram_tensor("x", [b, d, s], F16, kind="ExternalInput")
    wq_d = nc.dram_tensor("wq", [d, DHC], F16, kind="ExternalInput")
    wk_d = nc.dram_tensor("wk", [d, DHC], F16, kind="ExternalInput")
    wv_d = nc.dram_tensor("wv", [d, DHC], F16, kind="ExternalInput")
    bq_d = nc.dram_tensor("bq", [DHC, 1], F32, kind="ExternalInput")
    wo_d = nc.dram_tensor("wo", [DHC, d], F16, kind="ExternalInput")
    out_d = nc.dram_tensor("out", [b, s, d], F16, kind="ExternalOutput")

    with TileContext(nc) as tc:
        with (
            tc.tile_pool(name="const", bufs=1) as cpool,
            tc.tile_pool(name="wpool", bufs=3) as wpool,
            tc.tile_pool(name="xt", bufs=4) as xt_pool,
            tc.tile_pool(name="qk", bufs=4) as qk_pool,
            tc.tile_pool(name="vt", bufs=2) as vt_pool,
            tc.tile_pool(name="at", bufs=2) as at_pool,
            tc.tile_pool(name="pexp", bufs=3) as pexp_pool,
            tc.tile_pool(name="stg", bufs=4) as stg_pool,
            tc.tile_pool(name="small", bufs=8) as small,
            tc.tile_pool(name="osb", bufs=3) as osb_pool,
            tc.tile_pool(name="ps", bufs=1, space="PSUM") as pp,
        ):
            # ---- constants & weights ----
            # one batched DMA per weight tensor (issue cost on the sync
            # queue is ~600ns each; per-kt loads would serialize ~15us
            # ahead of the x prefetch). Order: wk, wv before x (the
            # prologue's first bursts need them), wq and the rest after.
            ones_col = cpool.tile([128, 32], F16, tag="ones_col")
            nc.vector.memset(ones_col[:, :], 1.0)
            w16 = {}
            walls = {}
            for name, dram in (("k", wk_d), ("v", wv_d), ("q", wq_d)):
                wall = wpool.tile([128, n_kt * DHC], F16, tag="w",
                                  name=f"w_{name}")
                walls[name] = (wall, dram)
                for kt in range(n_kt):
                    w16[(name, kt)] = wall[:, kt * DHC:(kt + 1) * DHC]

            def w_dma(name):
                wall, dram = walls[name]
                nc.sync.dma_start(
                    out=wall.rearrange("p (kt c) -> p kt c", kt=n_kt),
                    in_=dram.rearrange("(kt p) c -> p kt c", p=128),
                )

            w_dma("k")
            w_dma("v")
            w_dma("q")
            bq = cpool.tile([DHC, 1], F32, tag="bq")
            nc.sync.dma_start(out=bq[:, :], in_=bq_d[:, :])
            wo = cpool.tile([DHC, d], F16, tag="wo")

            # ---------- stage B builders (projections for one batch) ----
            def xT_dma(bi, xTs):
                # two supertile DMAs (4 kt-tiles each) instead of eight:
                # each dma_start costs ~600ns of sync-queue issue time
                for half in range(2):
                    nc.sync.dma_start(
                        out=xTs[0][half].rearrange("p (kt s) -> p kt s",
                                                   kt=4),
                        in_=x_d[bi, half * 512:(half + 1) * 512, :]
                        .rearrange("(kt p) s -> p kt s", p=128),
                    )

            def qk_burst(name, c, xT, dst, w=512):
                """One w-col chunk of the Q^T or K^T projection."""
                cs = slice(c * w, (c + 1) * w)
                ppr = pp.tile([128, w], F32, tag="scr", bufs=2, name="ppr")
                for kt in range(n_kt):
                    nc.tensor.matmul(
                        ppr[:, :], w16[(name, kt)][:, :], xT[1][kt][:, cs],
                        start=(kt == 0), stop=(kt == n_kt - 1),
                    )
                if name == "q":
                    nc.vector.tensor_scalar_add(dst[:, cs], ppr[:, :],
                                                bq[:, 0:1])
                else:
                    nc.vector.tensor_copy(dst[:, cs], ppr[:, :])

            def v_burst(tt, xT, vt):
                """One 128-token tile of V in natural [tok, dh] layout."""
                pv = pp.tile([128, 128], F32, tag="scr", bufs=2, name="pv")
                for kt in range(n_kt):
                    nc.tensor.matmul(
                        pv[:, :], xT[1][kt][:, tt * 128:(tt + 1) * 128],
                        w16[("v", kt)][:, :],
                        start=(kt == 0), stop=(kt == n_kt - 1),
                    )
                vdst = vt.rearrange("p (t two sv) -> p t two sv",
                                    two=2, sv=65)[:, tt, :, 0:64]
                nc.vector.tensor_copy(
                    vdst, pv.rearrange("p (two sv) -> p two sv", two=2)
                )

            def ones_fill(vt):
                ones_dst = vt.rearrange("p (t two sv) -> p t two sv",
                                        two=2, sv=65)[:, :, :, 64]
                nc.vector.tensor_copy(
                    ones_dst,
                    ones_col[:, 0:2 * n_tt]
                    .rearrange("p (t two) -> p t two", two=2))

            def make_b_work(xT, qT, kT, vt):
                """Stage-B work items (~1.3-1.7us of PE each), ordered so
                anything a batch's jc0 needs comes first (leftovers may
                spill into that batch's own early steps)."""
                work = []

                def qk(n, c, dst):
                    work.append(lambda: qk_burst(n, c, xT, dst))

                for c in range(4):
                    qk("k", c, kT)
                qk("q", 0, qT)
                for tt in range(n_tt):
                    work.append(lambda tt=tt: v_burst(tt, xT, vt))
                for c in range(1, 4):
                    qk("q", c, qT)
                return work

            # ---------- stage D (output projection units) ----------
            # d_stage holds the most recent q-chunk's units; they move to
            # d_queue one jc later so the PE never waits on the (slow-ish)
            # reciprocal -> broadcast -> normalize evacuation chain.
            d_queue = []   # (attnT, bi, tt, half)
            d_stage = []

            def emit_d_unit():
                if not d_queue:
                    return
                attnT_p, bi_out, tt, half = d_queue.pop(0)
                cs = slice(half * 512, (half + 1) * 512)
                po = pp.tile([128, 512], F32, tag="scr", bufs=2, name="po")
                nc.tensor.matmul(
                    po[:, :], attnT_p[:, tt * 128:(tt + 1) * 128],
                    wo[:, cs], start=True, stop=True,
                )
                osb = osb_pool.tile([128, 512], F16, tag="osb", name="osb")
                nc.vector.tensor_copy(osb[:, :], po[:, :])
                nc.sync.dma_start(
                    out=out_d[bi_out, tt * 128:(tt + 1) * 128, cs],
                    in_=osb[:, :],
                )

            # ---------- prologue: batch 0 stage A + leading stage B ----------
            def new_bufs():
                # xT lives as two 4-kt supertiles; per-kt views for compute
                sup = [xt_pool.tile([128, 4 * s], F16, tag="xt",
                                    name=f"xsup{h}") for h in range(2)]
                views = [sup[kt // 4][:, (kt % 4) * s:(kt % 4 + 1) * s]
                         for kt in range(n_kt)]
                qT = qk_pool.tile([DHC, s], F16, tag="qT")
                kT = qk_pool.tile([DHC, s], F16, tag="kT")
                vt = vt_pool.tile([128, n_tt * 130], F16, tag="vt")
                return (sup, views), qT, kT, vt

            cur = new_bufs()
            xT_dma(0, cur[0])
            nc.sync.dma_start(out=wo[:, :], in_=wo_d[:, :])
            ones_fill(cur[3])
            # minimum stage B to start jc=0: k/q chunk 0 and V tiles 0..2;
            # the rest streams in through b0_slots during jc0/jc1.
            qk_burst("k", 0, cur[0], cur[2])
            for tt in range(3):
                v_burst(tt, cur[0], cur[3])
            qk_burst("q", 0, cur[0], cur[1])
            _b0 = [("k", 1), ("v", 3), ("v", 4), ("v", 5), ("k", 2),
                   ("v", 6), ("v", 7), ("v", 8), ("k", 3), ("v", 9),
                   ("q", 1), ("v", 10), ("v", 11), ("v", 12), ("v", 13),
                   ("v", 14), ("v", 15), ("q", 2), ("q", 3)]
            b0_slots = {i + 1: [it] for i, it in enumerate(_b0)}

            def emit_b0(item):
                kind, i = item
                if kind == "v":
                    v_burst(i, cur[0], cur[3])
                else:
                    qk_burst(kind, i, cur[0],
                             cur[2] if kind == "k" else cur[1])

            # ---------- main loop: one continuous score-step stream ----------
            # Global step G runs (bi, jc, kt) = (G//64, (G%64)//16, G%16).
            # Scores for chunk jc+1 (or the next batch) start immediately
            # after chunk jc's - no pipeline drain at chunk boundaries, so
            # the scalar engine's exp stream never runs dry. AV matmuls lag
            # LAG steps behind; the av banks are released by a single CAST
            # into an SBUF staging tile, and normalization happens off the
            # critical path (recip -> gpsimd broadcast -> multiply).
            LAG = 2
            SPB = n_jc * n_tt          # score steps per batch = 64
            NG = b * SPB
            bufs_of = {0: cur}
            work_now = []              # leftovers for the current batch
            work_next = []             # stage B of the next batch (gated)
            stps = {}
            pexps = {}
            avs = {}

            def evac_jc(bi_, jc_):
                """Release av banks and queue stage D for (bi_, jc_)."""
                attnT = attnTs[bi_]
                qs = slice(jc_ * qw, (jc_ + 1) * qw)
                for h in range(HPC):
                    hs = slice(h * 64, (h + 1) * 64)
                    av = avs.pop((bi_, jc_, h))
                    stg = stg_pool.tile([65, qw], F32, tag="stg",
                                        name="stg")
                    nc.vector.tensor_copy(stg[:, :], av[:, :])
                    # off-path: normalize into attnT
                    rrow = small.tile([1, qw], F32, tag="rrow", name="rrow")
                    nc.vector.tensor_copy(rrow[:, :], stg[64:65, :])
                    recip = small.tile([1, qw], F32, tag="recip",
                                       name="recip")
                    nc.vector.reciprocal_approx_fast(recip[:, :], rrow[:, :])
                    rb = small.tile([64, qw], F32, tag="rb", name="rb")
                    nc.gpsimd.partition_broadcast(rb[:, :], recip[0:1, :])
                    nc.vector.tensor_mul(attnT[hs, qs], stg[0:64, :],
                                         rb[:, :])
                units = [(attnT, bi_, tt, half)
                         for tt in range(jc_ * (qw // 128),
                                         (jc_ + 1) * (qw // 128))
                         for half in range(2)]
                d_queue.extend(d_stage)
                d_stage.clear()
                if bi_ == b - 1:
                    d_queue.extend(units)
                else:
                    d_stage.extend(units)

            attnTs = {}
            for G in range(NG + LAG):
                if G < NG:
                    bi, rem = divmod(G, SPB)
                    jc, kt = divmod(rem, n_tt)
                    if rem == 0:
                        # batch bookkeeping
                        work_now = work_next
                        work_next = []
                        if bi + 1 < b:
                            bufs_of[bi + 1] = new_bufs()
                            work_next = make_b_work(*bufs_of[bi + 1])
                        attnTs[bi] = at_pool.tile([DHC, s], F16, tag="attnT",
                                                  name="attnT")
                    xT, qT, kT, vt = bufs_of[bi]
                    if kt == 0:
                        for h in range(HPC):
                            avs[(bi, jc, h)] = pp.tile(
                                [65, qw], F32, tag="av", bufs=2,
                                name=f"av{h}")
                    qs = slice(jc * qw, (jc + 1) * qw)
                    # score pair: h0 -> cols 0:512 (bank A), h1 -> B
                    stp = pp.tile([128, 2 * qw], F32, tag="stp",
                                  bufs=2, name="stp")
                    ks = slice(kt * 128, (kt + 1) * 128)
                    nc.tensor.matmul(
                        stp[:, 0:qw], kT[0:64, ks], qT[0:64, qs],
                        start=True, stop=True, tile_position=(0, 0),
                    )
                    nc.tensor.matmul(
                        stp[:, qw:2 * qw], kT[64:128, ks], qT[64:128, qs],
                        start=True, stop=True, tile_position=(64, 0),
                    )
                    stps[G] = stp
                    # interleave: DMA prefetch / stage D / stage B
                    if rem == 1 and bi + 1 < b:
                        xT_dma(bi + 1, bufs_of[bi + 1][0])
                        ones_fill(bufs_of[bi + 1][3])
                    if bi == 0 and rem in b0_slots:
                        for item in b0_slots.pop(rem):
                            emit_b0(item)
                    elif work_now:
                        work_now.pop(0)()
                    elif rem % 2 == 0 and d_queue:
                        emit_d_unit()
                    elif rem >= 13 and work_next:
                        work_next.pop(0)()
                    # exp for both heads in one ACTIVATE
                    pexp = pexp_pool.tile([128, 2 * qw], F16, tag="pexp",
                                          name="pexp", bufs=LAG + 2)
                    nc.scalar.activation(
                        pexp[:, :], stps[G][:, :], AF.Exp, scale=0.125,
                    )
                    pexps[G] = pexp
                Gp = G - LAG
                if Gp >= 0:
                    bip, remp = divmod(Gp, SPB)
                    jcp, ktp = divmod(remp, n_tt)
                    px = pexps.pop(Gp)
                    stps.pop(Gp, None)
                    vtv = bufs_of[bip][3].rearrange(
                        "p (t two sv) -> p t two sv", two=2, sv=65)
                    for h in range(HPC):
                        nc.tensor.matmul(
                            avs[(bip, jcp, h)][:, :], vtv[:, ktp, h, :],
                            px[:, h * qw:(h + 1) * qw],
                            start=(ktp == 0), stop=(ktp == n_tt - 1),
                        )
                    if ktp == n_tt - 1:
                        evac_jc(bip, jcp)
                        if remp == SPB - 1 and bip > 0:
                            bufs_of.pop(bip - 1, None)

            # flush remaining stage D
            d_queue.extend(d_stage)
            d_stage.clear()
            while d_queue:
                emit_d_unit()

    nc.compile()
    return nc


def _get_nc(b, s):
    key = (b, s)
    if key not in _CACHE:
        _CACHE[key] = build_nc(b, s)
    return _CACHE[key]


def make_in_maps(x, w_q, b_q, w_k, w_v, w_o):
    x16 = np.ascontiguousarray(
        np.asarray(x, dtype=np.float16).transpose(0, 2, 1))
    wq16 = np.asarray(w_q, dtype=np.float16)
    wk16 = np.asarray(w_k, dtype=np.float16)
    wv16 = np.asarray(w_v, dtype=np.float16)
    wo16 = np.asarray(w_o, dtype=np.float16)
    in_maps = []
    for i in range(NCORES):
        cs = slice(i * DHC, (i + 1) * DHC)
        in_maps.append({
            "x": x16,
            "wq": np.ascontiguousarray(wq16[:, cs]),
            "wk": np.ascontiguousarray(wk16[:, cs]),
            "wv": np.ascontiguousarray(wv16[:, cs]),
            "bq": np.ascontiguousarray(b_q[cs, None], dtype=np.float32),
            "wo": np.ascontiguousarray(wo16[cs, :]),
        })
    return in_maps


def kernel(x, w_q, b_q, w_k, b_k, w_v, b_v, w_o, b_o, _trace=False):
    x = np.asarray(x, dtype=np.float32)
    nc = _get_nc(x.shape[0], x.shape[1])
    in_maps = make_in_maps(x, w_q, b_q, w_k, w_v, w_o)
    kw = {}
    if _trace:
        import tempfile
        kw = dict(trace=True, trace_cores=list(range(NCORES)),
                  tmpdir=tempfile.mkdtemp(prefix="mha_trace_"))
    res = bass_utils.run_bass_kernel_spmd(
        nc, in_maps, core_ids=list(range(NCORES)), **kw
    )
    out = np.zeros(x.shape, dtype=np.float32)
    for i in range(NCORES):
        out += np.asarray(res.results[i]["out"], dtype=np.float32)
    out += np.asarray(b_o, dtype=np.float32)[None, None, :]
    # b_k cancels in softmax (constant per q); b_v @ w_o added here
    out += (np.asarray(b_v, dtype=np.float32)
            @ np.asarray(w_o, dtype=np.float32))[None, None, :]
    if _trace:
        return out, res
    return out
